# revision 5
# baseline (speedup 1.0000x reference)
"""DBRX attention block (B=1, T=2048, D=6144, 48 q heads / 8 kv heads, RoPE,
clamp, causal) as a Bass/Tile kernel on 8 Trainium2 NeuronCores.

Sharding: tensor-parallel over heads. Core c owns q heads [6c, 6c+6) and kv
head c (GQA groups align exactly: q head i uses kv head i//6).

The end-to-end latency of this problem is dominated by host<->device byte
traffic through the dispatch path, not by device compute (~1 ms), so the
layout is chosen to minimize shipped bytes:
  - x is row-sharded across cores ([768, T] fp16 slice of xT each) and
    AllGathered on device into the full fp16 xT.
  - w_qkv / w_out are exactly partitioned (no replication) and shipped fp16.
  - Constant tables (RoPE cos/sin, pair-swap permutation, identity, ones)
    are inline_tensor consts baked into the NEFF - zero per-call bytes.
  - After attention, the per-core head outputs are AllGathered (fp16) and
    each core computes only its 768-row slice of the output projection,
    returning a [768, T] fp16 shard; the host concatenates.

On-chip layout is feature-major ([feature, T]) throughout, so every matmul
is a clean [128 x 128] x [128 x 512] PE op at full rate (fp16 projections,
fp32r attention internals). Matmuls are scheduled weight-stationary where
possible (one lhsT reused across 4 PSUM banks):
  - qkvT[f, t]  = sum_d wqkvT[d, f] * xT[d, t]       (PSUM-accumulated over d)
  - RoPE via a pair-swap permutation matmul + DVE combine with cos/sin tables
  - scoresT[k, q] = sum_h kT[h, k] * qT[h, q], exp on ACT (no max-subtraction:
    scores are bounded by clamp at +-8, empirically |s| < ~25, exp fits fp32)
  - attnT[h, q] = sum_k v[k, h] * expT[k, q]; softmax denominator from a
    ones-vector matmul, applied via reciprocal + DMA partition-broadcast
  - outT[o, t] = sum_f w_oT[f, o] * attnT_full[f, t]  (o in this core's slice)
"""

import math
import sys
from contextlib import ExitStack

import numpy as np

for _p in ("/opt/trn_rl_repo", "/root/.axon_site/_ro/trn_rl_repo"):
    if _p not in sys.path:
        sys.path.append(_p)

import concourse.bass as bass
import concourse.tile as tile
from concourse import bacc, mybir
from concourse.bass_utils import run_bass_kernel_spmd

F16 = mybir.dt.float16
F32 = mybir.dt.float32
F32R = mybir.dt.float32r
MULT = mybir.AluOpType.mult
ADD = mybir.AluOpType.add
MAX = mybir.AluOpType.max
MIN = mybir.AluOpType.min
BYPASS = mybir.AluOpType.bypass
EXP = mybir.ActivationFunctionType.Exp

N_CORES = 8
D_MODEL = 6144
N_HEADS = 48
N_KV_HEADS = 8
HEAD_DIM = 128
NQH = N_HEADS // N_CORES          # q heads per core = 6
T = 2048
CLAMP = 8.0
ROPE_BASE = 500000.0
SCALE = 1.0 / math.sqrt(HEAD_DIM)
MASK_FLOOR = -30000.0             # fp16-safe; exp(SCALE*(s+floor)) == 0 in f32


def _chunk_plan(kd, chunk):
    """Chunk sizes: small leading chunks so the PE starts early."""
    plan = []
    rest = kd
    if kd > 8:
        plan = [1, 2, 3]
        rest = kd - 6
    while rest > 0:
        c = min(chunk, rest)
        plan.append(c)
        rest -= c
    return plan


def _rope_tables(t):
    hd = HEAD_DIM
    inv = 1.0 / ROPE_BASE ** (np.arange(0, hd, 2, dtype=np.float64) / hd)
    pos = np.arange(t, dtype=np.float64)
    freqs = pos[None, :] * inv[:, None]                     # [hd/2, T]
    cos = np.cos(freqs)
    sin = np.sin(freqs)
    cosf = np.empty((hd, t), dtype=np.float32)
    sinf = np.empty((hd, t), dtype=np.float32)
    cosf[0::2] = cos
    cosf[1::2] = cos
    sinf[0::2] = -sin                                       # row 2i:   -sin
    sinf[1::2] = sin                                        # row 2i+1: +sin
    return cosf, sinf


def build_program(d_model=D_MODEL, t=T, nqh=NQH, n_cores=N_CORES, chunk=6,
                  reps=1, stop_after=None):
    """Build the per-core Bass program. Returns the compiled Bacc handle."""
    kd = d_model // 128            # contraction tiles for qkv projection
    tq = t // 512                  # 512-wide T quads
    tb = t // 128                  # 128-wide T blocks
    nf = nqh + 2                   # feature tiles: q heads + k + v
    qkv_cols = nf * 128
    rpc = d_model // n_cores       # output rows per core
    att_feat = nqh * n_cores * 128  # total attention features across cores
    kd3 = att_feat // 128          # contraction tiles for out projection
    nf3 = rpc // 128               # output feature tiles per core
    groups = [list(range(n_cores))]

    nc = bacc.Bacc("TRN2", target_bir_lowering=False, debug=False)

    x_sh = nc.dram_tensor("x_sh", [rpc, t], F16, kind="ExternalInput").ap()
    wqkvT = nc.dram_tensor("wqkvT", [d_model, qkv_cols], F16,
                           kind="ExternalInput").ap()
    w_oT = nc.dram_tensor("w_oT", [att_feat, rpc], F16,
                          kind="ExternalInput").ap()
    maskT = nc.dram_tensor("maskT", [4, 128, 512], F16,
                           kind="ExternalInput").ap()
    out_sh = nc.dram_tensor("out_sh", [rpc, t], F16, kind="ExternalOutput").ap()

    cosf_np, sinf_np = _rope_tables(t)
    cosf = nc.inline_tensor(cosf_np, name="cosf").ap()
    sinf = nc.inline_tensor(sinf_np, name="sinf").ap()
    pswap_np = np.zeros((128, 128), dtype=np.float32)
    for i in range(0, 128, 2):
        pswap_np[i, i + 1] = 1.0
        pswap_np[i + 1, i] = 1.0
    pswap = nc.inline_tensor(pswap_np, name="pswap").ap()
    ident = nc.inline_tensor(np.eye(128, dtype=np.float32), name="ident").ap()
    ones = nc.inline_tensor(np.ones((128, 1), np.float32), name="ones").ap()

    # DRAM bounce + gathered tensors for the collectives
    bx = nc.dram_tensor("bx", [rpc, t], F16).ap()
    gx = nc.dram_tensor("gx", [d_model, t], F16, addr_space="Shared").ap()
    battn = nc.dram_tensor("battn", [nqh * 128, t], F16).ap()
    gattn = nc.dram_tensor("gattn", [att_feat, t], F16,
                           addr_space="Shared").ap()

    def _body(tc):
        with ExitStack() as octx:
            consts = octx.enter_context(tc.tile_pool(name="consts", bufs=1))
            accp = octx.enter_context(tc.tile_pool(name="accp", bufs=max(nf, nf3)))

            sb_cos = consts.tile([128, t], F32, tag="cos")
            nc.sync.dma_start(sb_cos[:], cosf[:])
            sb_sin = consts.tile([128, t], F32, tag="sin")
            nc.sync.dma_start(sb_sin[:], sinf[:])
            sb_mask16 = consts.tile([128, 4, 512], F16, tag="mask16")
            nc.sync.dma_start(sb_mask16[:], maskT.transpose([1, 0, 2]))
            sb_mask = consts.tile([128, 4, 512], F32, tag="mask")
            nc.vector.tensor_copy(sb_mask[:], sb_mask16[:])
            sb_pswap = consts.tile([128, 128], F32R, tag="pswap")
            nc.gpsimd.dma_start(sb_pswap[:], pswap[:])
            sb_ident = consts.tile([128, 128], F32R, tag="ident")
            nc.gpsimd.dma_start(sb_ident[:], ident[:])
            sb_ones = consts.tile([128, 1], F32R, tag="ones")
            nc.gpsimd.dma_start(sb_ones[:], ones[:])

            def one_rep():
                # ---- Phase 0: AllGather the x shard into full fp16 xT ----
                nc.gpsimd.dma_start(bx[:], x_sh[:])
                nc.gpsimd.collective_compute(
                    "AllGather", BYPASS, replica_groups=groups,
                    ins=[bx.opt()], outs=[gx.opt()])

                acc = [accp.tile([128, t], F32R, tag="acc", name=f"acc{i}")
                       for i in range(nf)]

                # ------------- Phase 1: qkvT = wqkvT-tiles.T @ xT -------------
                # Weight-stationary: one w block feeds all t-quads (4 PSUM
                # banks).
                with ExitStack() as ctx:
                    xp = ctx.enter_context(tc.tile_pool(name="xp", bufs=10))
                    wp = ctx.enter_context(tc.tile_pool(name="wp", bufs=9))
                    psp = ctx.enter_context(tc.tile_pool(name="psp", bufs=2 * tq,
                                                         space="PSUM"))
                    plan = _chunk_plan(kd, chunk)
                    base = 0
                    for c, csz in enumerate(plan):
                        ks = list(range(base, base + csz))
                        base += csz
                        xts, wts = [], []
                        for k in ks:
                            xt = xp.tile([128, t], F16, tag="xt")
                            nc.gpsimd.dma_start(xt[:], gx[k * 128:(k + 1) * 128, :])
                            xts.append(xt)
                            wt = wp.tile([128, qkv_cols], F16, tag="wt")
                            nc.gpsimd.dma_start(wt[:],
                                                wqkvT[k * 128:(k + 1) * 128, :])
                            wts.append(wt)
                        for f in range(nf):
                            pss = [psp.tile([128, 512], F32, tag="ps",
                                            name=f"ps{q}") for q in range(tq)]
                            for i in range(len(ks)):
                                lhs = wts[i][:, f * 128:(f + 1) * 128]
                                for q in range(tq):
                                    nc.tensor.matmul(
                                        pss[q][:], lhs,
                                        xts[i][:, q * 512:(q + 1) * 512],
                                        start=(i == 0), stop=(i == len(ks) - 1),
                                    )
                            for q in range(tq):
                                sl = slice(q * 512, (q + 1) * 512)
                                if c == 0:
                                    nc.vector.tensor_copy(acc[f][:, sl], pss[q][:])
                                else:
                                    nc.vector.scalar_tensor_tensor(
                                        out=acc[f][:, sl], in0=pss[q][:],
                                        scalar=1.0, in1=acc[f][:, sl],
                                        op0=MULT, op1=ADD,
                                    )

                if stop_after == "p1":
                    ob0 = consts.tile([128, 512], F16, tag="dbg")
                    nc.vector.tensor_copy(ob0[:], acc[0][:, 0:512].bitcast(F32))
                    nc.sync.dma_start(out_sh[0:128, 0:512], ob0[:])
                    return
                # late pools: open only after phase 1 frees its SBUF; closed
                # after the attn AllGather so phase 3 can reuse the space
                att_ctx = ExitStack()
                attp = att_ctx.enter_context(tc.tile_pool(name="attp", bufs=nqh))
                vtp = att_ctx.enter_context(tc.tile_pool(name="vtp", bufs=1))
                cvt = att_ctx.enter_context(tc.tile_pool(name="cvt", bufs=2))
                attnT = [attp.tile([128, t], F32R, tag="attnT", name=f"attnT{i}")
                         for i in range(nqh)]
                v_t = vtp.tile([128, tb, 128], F32R, tag="v_t")

                # -------- Phase 1.5: RoPE + clamp + v transpose --------
                # Emission order matters (DVE is FIFO): k head first, then v
                # work, then q heads in pair order — the attention loop below
                # is hp-outer, so rope for later head pairs hides under the
                # PE work of earlier pairs.
                rope_ctx = ExitStack()
                # shared PSUM pool: rope pair-swap tiles + attention scores
                spp = rope_ctx.enter_context(tc.tile_pool(name="spp", bufs=4,
                                                          space="PSUM"))
                tmp = rope_ctx.enter_context(tc.tile_pool(name="rtmp", bufs=3))

                def emit_rope(f):
                    for q in range(tq):
                        sl = slice(q * 512, (q + 1) * 512)
                        pshuf = spp.tile([128, 512], F32, tag="ps", name="pshuf")
                        nc.tensor.matmul(pshuf[:], sb_pswap[:], acc[f][:, sl],
                                         start=True, stop=True)
                        t1 = tmp.tile([128, 512], F32, tag="t1", name="t1")
                        nc.vector.tensor_tensor(t1[:], acc[f][:, sl].bitcast(F32),
                                                sb_cos[:, sl], op=MULT)
                        t2 = tmp.tile([128, 512], F32, tag="t2", name="t2")
                        nc.vector.tensor_tensor(t2[:], pshuf[:], sb_sin[:, sl],
                                                op=MULT)
                        t3 = tmp.tile([128, 512], F32, tag="t3", name="t3")
                        nc.vector.tensor_tensor(t3[:], t1[:], t2[:], op=ADD)
                        nc.vector.tensor_scalar(acc[f][:, sl], t3[:], -CLAMP,
                                                CLAMP, op0=MAX, op1=MIN)

                def emit_vwork():
                    iv = nqh + 1                  # v: clamp only, then transpose
                    for q in range(tq):
                        sl = slice(q * 512, (q + 1) * 512)
                        t4 = tmp.tile([128, 512], F32, tag="t1", name="t1")
                        nc.vector.tensor_scalar(t4[:], acc[iv][:, sl].bitcast(F32),
                                                -CLAMP, CLAMP, op0=MAX, op1=MIN)
                        nc.vector.tensor_copy(acc[iv][:, sl], t4[:])
                    for j in range(tb):
                        pt = spp.tile([128, 128], F32R, tag="ps", name="pt")
                        nc.tensor.transpose(pt[:],
                                            acc[iv][:, j * 128:(j + 1) * 128],
                                            sb_ident[:])
                        nc.vector.tensor_copy(v_t[:, j, :], pt[:])

                emit_rope(nqh)                    # k head
                emit_vwork()

                def emit_attn_export(h):
                    # attnT[h] (f32r) -> fp16 -> battn rows for the AllGather
                    c16 = cvt.tile([128, t], F16, tag="c16", name="c16")
                    nc.vector.tensor_copy(c16[:], attnT[h][:].bitcast(F32))
                    nc.sync.dma_start(battn[h * 128:(h + 1) * 128, :], c16[:])

                # -------- Phase 2: causal attention (head pairs) --------
                # Software-pipelined: score matmuls + exp run PIPE kb-steps
                # ahead of the PV/denominator matmuls so the PE never waits on
                # the ACT round-trip. PSUM: 4 score banks + 2 attn banks + 2
                # sum banks. hp-outer so emit_rope(f) for pair p+1 overlaps
                # pair p's matmuls.
                PIPE = 2
                with ExitStack() as ctx:
                    exps = ctx.enter_context(tc.tile_pool(name="exps",
                                                          bufs=2 * (PIPE + 1)))
                    psa_p = ctx.enter_context(tc.tile_pool(name="psa", bufs=2,
                                                           space="PSUM"))
                    psm_p = ctx.enter_context(tc.tile_pool(name="psm", bufs=2,
                                                           space="PSUM"))
                    rcp = ctx.enter_context(tc.tile_pool(name="rcp", bufs=2))
                    rcb = ctx.enter_context(tc.tile_pool(name="rcb", bufs=2))
                    ik = nqh                      # k head feature tile
                    for hp in range(nqh // 2):
                        hs = (2 * hp, 2 * hp + 1)
                        emit_rope(hs[0])
                        emit_rope(hs[1])
                        for J in range(tq):
                            qsl = slice(J * 512, (J + 1) * 512)
                            nkb = 4 * J + 4
                            pa = {h: psa_p.tile([128, 512], F32, tag="pa",
                                                name="pa") for h in hs}
                            pm = {h: psm_p.tile([1, 512], F32, tag="pm",
                                                name="pm") for h in hs}
                            exq = {}
                            for kb in range(nkb + PIPE):
                                if kb < nkb:
                                    klhs = acc[ik][:, kb * 128:(kb + 1) * 128]
                                    for h in hs:
                                        ps = spp.tile([128, 512], F32, tag="ps",
                                                      name="ps")
                                        nc.tensor.matmul(ps[:], klhs,
                                                         acc[h][:, qsl],
                                                         start=True, stop=True)
                                        if kb >= 4 * J:
                                            nc.vector.tensor_tensor(
                                                ps[:], ps[:],
                                                sb_mask[:, kb - 4 * J, :], op=ADD)
                                        ex = exps.tile([128, 512], F32R, tag="ex",
                                                       name="ex")
                                        nc.scalar.activation(ex[:], ps[:], EXP,
                                                             scale=SCALE)
                                        exq[(kb, h)] = ex
                                kbd = kb - PIPE
                                if kbd < 0 or kbd >= nkb:
                                    continue
                                st = (kbd == 0)
                                sp = (kbd == nkb - 1)
                                for h in hs:
                                    nc.tensor.matmul(pa[h][:], v_t[:, kbd, :],
                                                     exq[(kbd, h)][:],
                                                     start=st, stop=sp)
                                for h in hs:
                                    nc.tensor.matmul(pm[h][:], sb_ones[:],
                                                     exq.pop((kbd, h))[:],
                                                     start=st, stop=sp)
                            for h in hs:
                                rc = rcp.tile([1, 512], F32R, tag="rc", name="rc")
                                nc.vector.reciprocal(rc[:], pm[h][:])
                                rb = rcb.tile([128, 512], F32R, tag="rb",
                                              name="rb")
                                nc.sync.dma_start(
                                    out=rb[:],
                                    in_=bass.AP(tensor=rc.tensor,
                                                offset=rc.offset,
                                                ap=[[1, 1], [0, 128]] + rc.ap[1:]))
                                nc.vector.tensor_tensor(attnT[h][:, qsl],
                                                        pa[h][:], rb[:], op=MULT)
                        # export this pair's finished heads while the next
                        # pair computes
                        emit_attn_export(hs[0])
                        emit_attn_export(hs[1])

                rope_ctx.close()

                # ---- Phase 2.5: AllGather all cores' attn outputs (fp16) ----
                nc.gpsimd.collective_compute(
                    "AllGather", BYPASS, replica_groups=groups,
                    ins=[battn.opt()], outs=[gattn.opt()])
                att_ctx.close()

                if stop_after == "attn":
                    ob0 = consts.tile([128, 512], F16, tag="dbg")
                    nc.vector.tensor_copy(ob0[:], attnT[0][:, 0:512].bitcast(F32))
                    nc.sync.dma_start(out_sh[0:128, 0:512], ob0[:])
                    return
                # ---- Phase 3: out projection (own 768-row slice) ----
                # Same chunked weight-stationary pattern as phase 1:
                # outT[o, t] = sum_f w_oT[f, o] * gattn[f, t], f over all
                # cores' heads.
                with ExitStack() as ctx:
                    xp3 = ctx.enter_context(tc.tile_pool(name="xp3", bufs=10))
                    wp3 = ctx.enter_context(tc.tile_pool(name="wp3", bufs=9))
                    ps3 = ctx.enter_context(tc.tile_pool(name="ps3", bufs=2 * tq,
                                                         space="PSUM"))
                    outp = ctx.enter_context(tc.tile_pool(name="outp", bufs=2))
                    oacc = [accp.tile([128, t], F32R, tag="acc", name=f"oacc{i}")
                            for i in range(nf3)]
                    plan = _chunk_plan(kd3, chunk)
                    base = 0
                    for c, csz in enumerate(plan):
                        ks = list(range(base, base + csz))
                        base += csz
                        ats, wts = [], []
                        for k in ks:
                            at = xp3.tile([128, t], F16, tag="at")
                            nc.gpsimd.dma_start(at[:],
                                                gattn[k * 128:(k + 1) * 128, :])
                            ats.append(at)
                            wt = wp3.tile([128, rpc], F16, tag="wt3")
                            nc.gpsimd.dma_start(wt[:],
                                                w_oT[k * 128:(k + 1) * 128, :])
                            wts.append(wt)
                        for f in range(nf3):
                            pss = [ps3.tile([128, 512], F32, tag="po",
                                            name=f"po{q}") for q in range(tq)]
                            for i in range(len(ks)):
                                lhs = wts[i][:, f * 128:(f + 1) * 128]
                                for q in range(tq):
                                    nc.tensor.matmul(
                                        pss[q][:], lhs,
                                        ats[i][:, q * 512:(q + 1) * 512],
                                        start=(i == 0), stop=(i == len(ks) - 1),
                                    )
                            for q in range(tq):
                                sl = slice(q * 512, (q + 1) * 512)
                                if c == 0:
                                    nc.vector.tensor_copy(oacc[f][:, sl],
                                                          pss[q][:])
                                else:
                                    nc.vector.scalar_tensor_tensor(
                                        out=oacc[f][:, sl], in0=pss[q][:],
                                        scalar=1.0, in1=oacc[f][:, sl],
                                        op0=MULT, op1=ADD,
                                    )
                    for f in range(nf3):
                        ob = outp.tile([128, t], F16, tag="ob", name="ob")
                        nc.vector.tensor_copy(ob[:], oacc[f][:].bitcast(F32))
                        nc.sync.dma_start(out_sh[f * 128:(f + 1) * 128, :], ob[:])

            for _ in range(reps):
                one_rep()

    with tile.TileContext(nc) as tc, nc.allow_low_precision(reason="fp16/fp32r matmuls"):
        _body(tc)
    nc.compile()
    return nc


def make_core_inputs(x, causal_mask, w_qkv, w_out, d_model=D_MODEL, t=T, nqh=NQH,
                     n_cores=N_CORES):
    """Host-side sharding: per-core input dicts for the SPMD program."""
    x2T = np.ascontiguousarray(x.reshape(t, d_model).T.astype(np.float16))
    rpc = d_model // n_cores
    att_feat = nqh * n_cores * 128
    kv_base = nqh * n_cores * HEAD_DIM
    n_kv = n_cores

    cm = causal_mask.reshape(causal_mask.shape[-2], causal_mask.shape[-1])
    maskT = np.stack([np.ascontiguousarray(cm[0:512, m * 128:(m + 1) * 128].T)
                      for m in range(4)])
    maskT = np.maximum(maskT, MASK_FLOOR).astype(np.float16)

    in_maps = []
    for c in range(n_cores):
        qrows = np.arange(c * nqh * 128, (c + 1) * nqh * 128)
        krows = np.arange(kv_base + c * 128, kv_base + (c + 1) * 128)
        vrows = np.arange(kv_base + n_kv * 128 + c * 128,
                          kv_base + n_kv * 128 + (c + 1) * 128)
        rows = np.concatenate([qrows, krows, vrows])
        wqkvT_c = np.ascontiguousarray(w_qkv[rows, :].T.astype(np.float16))
        w_oT_c = np.ascontiguousarray(
            w_out[c * rpc:(c + 1) * rpc, :att_feat].T.astype(np.float16))
        in_maps.append({
            "x_sh": x2T[c * rpc:(c + 1) * rpc],
            "wqkvT": wqkvT_c,
            "w_oT": w_oT_c,
            "maskT": maskT,
        })
    return in_maps


_PROGRAM_CACHE = {}


def _get_program():
    key = (D_MODEL, T, NQH)
    if key not in _PROGRAM_CACHE:
        _PROGRAM_CACHE[key] = build_program()
    return _PROGRAM_CACHE[key]


def kernel(x, causal_mask, w_qkv, w_out):
    x = np.asarray(x, dtype=np.float32)
    causal_mask = np.asarray(causal_mask, dtype=np.float32)
    w_qkv = np.asarray(w_qkv, dtype=np.float32)
    w_out = np.asarray(w_out, dtype=np.float32)

    nc = _get_program()
    in_maps = make_core_inputs(x, causal_mask, w_qkv, w_out)
    res = run_bass_kernel_spmd(nc, in_maps, list(range(N_CORES)))
    outT = np.concatenate([res.results[c]["out_sh"] for c in range(N_CORES)],
                          axis=0).astype(np.float32)
    return np.ascontiguousarray(outT.T).reshape(1, T, D_MODEL)
